# revision 5
# baseline (speedup 1.0000x reference)
"""AttnDecoderRNN step on 8 Trainium2 NeuronCores.

Sharding (tensor-parallel on vocab + hidden):
  - out_W/out_b: row(vocab)-sharded, 6283 rows/core (padded to 8*6283=50264).
    Host pre-transposes to [H, VS] so PE contracts over h on partitions.
  - comb_W: row-sharded (each core computes its 128-wide slice of x).
  - w_ih/w_hh: column-sharded (each core contracts over its 128-slice of
    x / h0); partial gi||gh all-reduced (24KB) across the 8 cores.
  - attention weights + encoder_outputs + biases: replicated (tiny).
  - emb_W: replicated to all cores; token row gathered on-device via
    indirect DMA.
  - log-softmax: each core computes its logits shard + local (max, sumexp);
    one 64B AllGather combines stats; logp shard written per core.

Layout: all activation vectors live as column-chunk tiles [128, nchunks]
(v[c*128+p] at [p, c]) so every matvec is a PE matmul with a [128,1] rhs or
lhsT and no transposes are needed (except two tiny PE transposes for the
attention weight row).
"""

import os
import sys
import time

import numpy as np

sys.path.insert(0, "/opt/trn_rl_repo")

H = 1024
V = 50257
L = 150  # encoder length
NCORES = 8
VS = 6283  # per-core vocab shard (8*6283 = 50264 >= 50257)
HC = H // 128  # 8 h-chunks
CC = 2 * H // 128  # 16 chunks of the 2H concat vectors
GC = 3 * H // 128  # 24 gate-output chunks
NT = 512  # vocab tile (free dim) for the big matmul
NEG_BIG = -1.0e30

_CACHE = {}


def _build(out_dt_name: str, nbufs: int):
    import concourse.bacc as bacc
    import concourse.tile as tile
    from concourse import bass, mybir

    f32 = mybir.dt.float32
    i32 = mybir.dt.int32
    w_dt = getattr(mybir.dt, out_dt_name)

    nc = bacc.Bacc(
        "TRN2",
        target_bir_lowering=False,
        debug=False,
        enable_asserts=True,
        num_devices=NCORES,
    )

    def inp(name, shape, dt=f32):
        return nc.dram_tensor(name, list(shape), dt, kind="ExternalInput").ap()

    def outp(name, shape, dt=f32):
        return nc.dram_tensor(name, list(shape), dt, kind="ExternalOutput").ap()

    # ---- inputs (per-core views prepared on host) ----
    tok2_d = inp("tok2", [2, 1], i32)            # token index, replicated x2
    embw_d = inp("emb_w", [V, H])                # full embedding (replicated)
    hcols_d = inp("h_cols", [128, HC])           # h0 column-chunks
    h0s_d = inp("h0_slice", [128, 1])            # this core's h0 slice
    enc0_d = inp("enc0", [128, H])               # encoder rows 0..127
    enc1_d = inp("enc1", [L - 128, H])           # encoder rows 128..149
    awt_d = inp("attn_wt", [128, CC * L])        # attn_W.T chunk-packed
    attb_d = inp("attn_b", [1, L])
    cwt_d = inp("comb_wt", [128, CC * 128])      # comb_W.T shard chunk-packed
    cb_d = inp("comb_b", [128, 1])               # comb_b slice (column)
    iwt_d = inp("wih_t", [128, 3 * H])           # w_ih.T rows-slice
    hwt_d = inp("whh_t", [128, 3 * H])           # w_hh.T rows-slice
    bi_d = inp("bi_cols", [128, GC])             # b_ih column-chunks
    bh_d = inp("bh_cols", [128, GC])             # b_hh column-chunks
    owt_d = inp("out_wt", [H, VS], w_dt)         # out_W.T shard
    ob_d = inp("out_b", [1, VS])                 # out_b shard (pad NEG_BIG)

    # ---- outputs ----
    logp_d = outp("logp_s", [VS])
    hnew_d = outp("hnew_cols", [128, HC])
    aw_d = outp("aw_out", [L])

    vtiles = []
    v0 = 0
    while v0 < VS:
        vtiles.append((v0, min(NT, VS - v0)))
        v0 += NT
    n_vt = len(vtiles)

    X = mybir.AxisListType.X
    AF = mybir.ActivationFunctionType
    OP = mybir.AluOpType

    with tile.TileContext(nc) as tc:
        with (
            tc.tile_pool(name="consts", bufs=1) as cpool,
            tc.tile_pool(name="wstream", bufs=nbufs) as spool,
            tc.tile_pool(name="psmall", bufs=2, space="PSUM") as psmall,
            tc.tile_pool(name="pgate", bufs=1, space="PSUM") as pgate,
            tc.tile_pool(name="pbig", bufs=2, space="PSUM") as pbig,
            tc.tile_pool(name="dram", bufs=1, space="DRAM") as dpool,
        ):
            # --- small constant loads ---
            ident = cpool.tile([1, 1], f32, name="ident")
            nc.vector.memset(ident[:], 1.0)

            idx_t = cpool.tile([2, 1], i32, name="idx_t")
            nc.sync.dma_start(out=idx_t[:], in_=tok2_d[:, :])

            h_cols = cpool.tile([128, HC], f32, name="h_cols_t")
            nc.sync.dma_start(out=h_cols[:], in_=hcols_d[:, :])
            h0s_t = cpool.tile([128, 1], f32, name="h0s_t")
            nc.sync.dma_start(out=h0s_t[:], in_=h0s_d[:, :])

            enc0_t = cpool.tile([128, H], f32, name="enc0_t")
            nc.sync.dma_start(out=enc0_t[:], in_=enc0_d[:, :])
            enc1_t = cpool.tile([L - 128, H], f32, name="enc1_t")
            nc.sync.dma_start(out=enc1_t[:], in_=enc1_d[:, :])

            awt_t = cpool.tile([128, CC * L], f32, name="awt_t")
            nc.sync.dma_start(out=awt_t[:], in_=awt_d[:, :])
            attb_t = cpool.tile([1, L], f32, name="attb_t")
            nc.sync.dma_start(out=attb_t[:], in_=attb_d[:, :])

            cwt_t = cpool.tile([128, CC * 128], f32, name="cwt_t")
            nc.sync.dma_start(out=cwt_t[:], in_=cwt_d[:, :])
            cb_t = cpool.tile([128, 1], f32, name="cb_t")
            nc.sync.dma_start(out=cb_t[:], in_=cb_d[:, :])

            iwt_t = cpool.tile([128, 3 * H], f32, name="iwt_t")
            nc.sync.dma_start(out=iwt_t[:], in_=iwt_d[:, :])
            hwt_t = cpool.tile([128, 3 * H], f32, name="hwt_t")
            nc.sync.dma_start(out=hwt_t[:], in_=hwt_d[:, :])

            bi_t = cpool.tile([128, GC], f32, name="bi_t")
            nc.sync.dma_start(out=bi_t[:], in_=bi_d[:, :])
            bh_t = cpool.tile([128, GC], f32, name="bh_t")
            nc.sync.dma_start(out=bh_t[:], in_=bh_d[:, :])

            ob_t = cpool.tile([1, VS], f32, name="ob_t")
            nc.sync.dma_start(out=ob_t[:], in_=ob_d[:, :])

            # --- embedding gather: row idx of emb_W -> column chunks ---
            erow = cpool.tile([2, H], f32, name="erow")
            nc.gpsimd.indirect_dma_start(
                out=erow[:],
                out_offset=None,
                in_=embw_d[:, :],
                in_offset=bass.IndirectOffsetOnAxis(ap=idx_t[:, 0:1], axis=0),
            )
            emb_scr = dpool.tile([H], f32, name="emb_scr")
            nc.sync.dma_start(out=emb_scr[:], in_=erow[0:1, :])
            e_cols = cpool.tile([128, HC], f32, name="e_cols")
            nc.sync.dma_start(
                out=e_cols[:], in_=emb_scr[:].rearrange("(c p) -> p c", p=128)
            )

            def cat_col(c):
                # column c of the [embedded | h0-or-attn] concat vectors
                return e_cols[:, c : c + 1] if c < HC else None

            # --- attention scores: s[1,L] = attn_in @ attn_W.T ---
            ps_s = psmall.tile([1, L], f32, name="ps_s", tag="pss")
            for c in range(CC):
                lhs = (
                    e_cols[:, c : c + 1]
                    if c < HC
                    else h_cols[:, c - HC : c - HC + 1]
                )
                nc.tensor.matmul(
                    ps_s[:, :],
                    lhsT=lhs,
                    rhs=awt_t[:, c * L : (c + 1) * L],
                    start=(c == 0),
                    stop=(c == CC - 1),
                )

            s_sb = cpool.tile([1, L], f32, name="s_sb")
            nc.vector.tensor_add(s_sb[:], ps_s[:], attb_t[:])
            nmax_s = cpool.tile([1, 1], f32, name="nmax_s")
            nc.vector.reduce_max(nmax_s[:], s_sb[:], axis=X, negate=True)
            exps = cpool.tile([1, L], f32, name="exps")
            ssum = cpool.tile([1, 1], f32, name="ssum")
            nc.scalar.activation(
                out=exps[:], in_=s_sb[:], func=AF.Exp,
                bias=nmax_s[:, 0:1], accum_out=ssum[:, 0:1],
            )
            rsum = cpool.tile([1, 1], f32, name="rsum")
            nc.vector.reciprocal(rsum[:], ssum[:])
            aw_t = cpool.tile([1, L], f32, name="aw_t")
            nc.vector.tensor_scalar_mul(aw_t[:], exps[:], rsum[:, 0:1])
            nc.sync.dma_start(out=aw_d[:], in_=aw_t[0:1, :])

            # --- attn weights row -> columns (two PE transposes) ---
            ps_t0 = psmall.tile([128, 1], f32, name="ps_t0", tag="pss")
            nc.tensor.transpose(ps_t0[:], aw_t[0:1, 0:128], ident[0:1, 0:1])
            awc0 = cpool.tile([128, 1], f32, name="awc0")
            nc.scalar.copy(awc0[:], ps_t0[:])
            ps_t1 = psmall.tile([L - 128, 1], f32, name="ps_t1", tag="pss")
            nc.tensor.transpose(ps_t1[:], aw_t[0:1, 128:L], ident[0:1, 0:1])
            awc1 = cpool.tile([L - 128, 1], f32, name="awc1")
            nc.scalar.copy(awc1[:], ps_t1[:])

            # --- attn_applied columns: a[:,c] = enc[:, c-slice].T @ aw ---
            a_cols = cpool.tile([128, HC], f32, name="a_cols")
            for c in range(HC):
                ps_a = psmall.tile([128, 1], f32, name="ps_a", tag="pss")
                nc.tensor.matmul(
                    ps_a[:], lhsT=enc0_t[:, c * 128 : (c + 1) * 128],
                    rhs=awc0[:], start=True, stop=False,
                )
                nc.tensor.matmul(
                    ps_a[:], lhsT=enc1_t[:, c * 128 : (c + 1) * 128],
                    rhs=awc1[:], start=False, stop=True,
                )
                nc.scalar.copy(a_cols[:, c : c + 1], ps_a[:])

            # --- comb: x_col = relu(comb_W_slice @ comb_in + comb_b) ---
            ps_x = pgate.tile([128, 1], f32, name="ps_x")
            for c in range(CC):
                rhs = (
                    e_cols[:, c : c + 1]
                    if c < HC
                    else a_cols[:, c - HC : c - HC + 1]
                )
                nc.tensor.matmul(
                    ps_x[:], lhsT=cwt_t[:, c * 128 : (c + 1) * 128],
                    rhs=rhs, start=(c == 0), stop=(c == CC - 1),
                )
            x_col = cpool.tile([128, 1], f32, name="x_col")
            nc.scalar.activation(
                out=x_col[:], in_=ps_x[:], func=AF.Relu, bias=cb_t[:, 0:1]
            )

            # --- GRU partial pre-activations (this core's 128-slice of x/h0) ---
            ps_gi = pgate.tile([128, GC], f32, name="ps_gi")
            ps_gh = pgate.tile([128, GC], f32, name="ps_gh")
            for c in range(GC):
                nc.tensor.matmul(
                    ps_gi[:, c : c + 1], lhsT=iwt_t[:, c * 128 : (c + 1) * 128],
                    rhs=x_col[:], start=True, stop=True,
                )
            for c in range(GC):
                nc.tensor.matmul(
                    ps_gh[:, c : c + 1], lhsT=hwt_t[:, c * 128 : (c + 1) * 128],
                    rhs=h0s_t[:], start=True, stop=True,
                )
            cc_sb = cpool.tile([128, 2 * GC], f32, name="cc_sb")
            nc.vector.tensor_copy(cc_sb[:, 0:GC], ps_gi[:])
            nc.vector.tensor_copy(cc_sb[:, GC : 2 * GC], ps_gh[:])

            cc_in = dpool.tile([128, 2 * GC], f32, name="cc_in")
            cc_out = dpool.tile(
                [128, 2 * GC], f32, name="cc_out", addr_space="Shared"
            )
            nc.sync.dma_start(out=cc_in[:, :], in_=cc_sb[:])
            nc.gpsimd.collective_compute(
                "AllReduce",
                OP.add,
                replica_groups=[list(range(NCORES))],
                ins=[cc_in[:, :]],
                outs=[cc_out[:, :]],
            )
            g_t = cpool.tile([128, 2 * GC], f32, name="g_t")
            nc.sync.dma_start(out=g_t[:], in_=cc_out[:, :])

            # --- gates ([128, HC] column layout; r|z|n at cols 0-7|8-15|16-23) ---
            bsum = cpool.tile([128, 16], f32, name="bsum")
            nc.vector.tensor_add(bsum[:], bi_t[:, 0:16], bh_t[:, 0:16])
            rz1 = cpool.tile([128, 16], f32, name="rz1")
            nc.vector.tensor_add(rz1[:], g_t[:, 0:16], g_t[:, GC : GC + 16])
            rz2 = cpool.tile([128, 16], f32, name="rz2")
            nc.vector.tensor_add(rz2[:], rz1[:], bsum[:])
            rz = cpool.tile([128, 16], f32, name="rz")
            nc.scalar.activation(out=rz[:], in_=rz2[:], func=AF.Sigmoid)

            t1 = cpool.tile([128, HC], f32, name="t1")
            nc.vector.tensor_add(
                t1[:], g_t[:, GC + 16 : GC + 24], bh_t[:, 16:24]
            )
            t2 = cpool.tile([128, HC], f32, name="t2")
            nc.vector.tensor_mul(t2[:], rz[:, 0:8], t1[:])
            t3 = cpool.tile([128, HC], f32, name="t3")
            nc.vector.tensor_add(t3[:], g_t[:, 16:24], bi_t[:, 16:24])
            t4 = cpool.tile([128, HC], f32, name="t4")
            nc.vector.tensor_add(t4[:], t3[:], t2[:])
            n_t = cpool.tile([128, HC], f32, name="n_t")
            nc.scalar.activation(out=n_t[:], in_=t4[:], func=AF.Tanh)

            d1 = cpool.tile([128, HC], f32, name="d1")
            nc.vector.tensor_sub(d1[:], h_cols[:], n_t[:])
            d2 = cpool.tile([128, HC], f32, name="d2")
            nc.vector.tensor_mul(d2[:], rz[:, 8:16], d1[:])
            hN = cpool.tile([128, HC], f32, name="hN")
            nc.vector.tensor_add(hN[:], n_t[:], d2[:])
            nc.sync.dma_start(out=hnew_d[:, :], in_=hN[:])

            if out_dt_name == "float32":
                hN_mm = hN
            else:
                hN_mm = cpool.tile([128, HC], w_dt, name="hN_mm")
                nc.vector.tensor_copy(hN_mm[:], hN[:])

            # --- big matmul: logits shard = h_new @ out_W_shard.T + out_b ---
            logits = cpool.tile([1, VS], f32, name="logits")
            nmaxs = cpool.tile([1, n_vt], f32, name="nmaxs")
            for t, (v0, ntv) in enumerate(vtiles):
                ps_l = pbig.tile([1, NT], f32, name="ps_l")
                for c in range(HC):
                    wt = spool.tile([128, NT], w_dt, name="wt")
                    nc.sync.dma_start(
                        out=wt[:, :ntv],
                        in_=owt_d[c * 128 : (c + 1) * 128, v0 : v0 + ntv],
                    )
                    nc.tensor.matmul(
                        ps_l[:, :ntv],
                        lhsT=hN_mm[:, c : c + 1],
                        rhs=wt[:, :ntv],
                        start=(c == 0),
                        stop=(c == HC - 1),
                    )
                nc.vector.tensor_add(
                    logits[0:1, v0 : v0 + ntv], ps_l[0:1, :ntv],
                    ob_t[0:1, v0 : v0 + ntv],
                )
                nc.vector.reduce_max(
                    nmaxs[0:1, t : t + 1], logits[0:1, v0 : v0 + ntv],
                    axis=X, negate=True,
                )

            # --- local softmax stats ---
            nM = cpool.tile([1, 1], f32, name="nM")
            nc.vector.tensor_reduce(
                nM[:], nmaxs[0:1, 0:n_vt], axis=X, op=OP.min
            )
            rowbuf = cpool.tile([1, VS], f32, name="rowbuf")
            S_t = cpool.tile([1, 1], f32, name="S_t")
            nc.scalar.activation(
                out=rowbuf[:], in_=logits[:], func=AF.Exp,
                bias=nM[:, 0:1], accum_out=S_t[:, 0:1],
            )
            st = cpool.tile([1, 2], f32, name="st")
            nc.vector.tensor_scalar_mul(st[:, 0:1], nM[:], -1.0)
            nc.vector.tensor_copy(st[:, 1:2], S_t[:])

            st_in = dpool.tile([1, 2], f32, name="st_in")
            st_out = dpool.tile(
                [NCORES, 2], f32, name="st_out", addr_space="Shared"
            )
            nc.sync.dma_start(out=st_in[:, :], in_=st[0:1, :])
            nc.gpsimd.collective_compute(
                "AllGather",
                OP.bypass,
                replica_groups=[list(range(NCORES))],
                ins=[st_in[:, :]],
                outs=[st_out[:, :]],
            )
            ms_t = cpool.tile([1, NCORES], f32, name="ms_t")
            nc.sync.dma_start(
                out=ms_t[:], in_=st_out[:, 0:1].rearrange("k one -> one k")
            )
            ss_t = cpool.tile([1, NCORES], f32, name="ss_t")
            nc.sync.dma_start(
                out=ss_t[:], in_=st_out[:, 1:2].rearrange("k one -> one k")
            )

            # --- global stats + logp ---
            nMg = cpool.tile([1, 1], f32, name="nMg")
            nc.vector.reduce_max(nMg[:], ms_t[:], axis=X, negate=True)
            arg = cpool.tile([1, NCORES], f32, name="arg")
            nc.vector.tensor_scalar_add(arg[:], ms_t[:], nMg[:, 0:1])
            eg = cpool.tile([1, NCORES], f32, name="eg")
            nc.scalar.activation(out=eg[:], in_=arg[:], func=AF.Exp)
            terms = cpool.tile([1, NCORES], f32, name="terms")
            nc.vector.tensor_mul(terms[:], eg[:], ss_t[:])
            Sg = cpool.tile([1, 1], f32, name="Sg")
            nc.vector.reduce_sum(Sg[:], terms[:], axis=X)
            lnS = cpool.tile([1, 1], f32, name="lnS")
            nc.scalar.activation(out=lnS[:], in_=Sg[:], func=AF.Ln)
            noff = cpool.tile([1, 1], f32, name="noff")
            nc.vector.tensor_sub(noff[:], nMg[:], lnS[:])
            nc.scalar.activation(
                out=rowbuf[:], in_=logits[:], func=AF.Identity,
                bias=noff[:, 0:1], scale=1.0,
            )
            nc.sync.dma_start(out=logp_d[:], in_=rowbuf[0:1, :])

    nc.compile()
    return nc


def _get_nc(out_dt_name, nbufs):
    key = (out_dt_name, nbufs)
    if key not in _CACHE:
        _CACHE[key] = _build(out_dt_name, nbufs)
    return _CACHE[key]


OUT_DT = "float32"
NBUFS = 28


def _make_in_maps(inputs):
    f4 = np.float32
    idx = np.asarray(inputs["input"]).reshape(-1)[0]
    tok2 = np.full((2, 1), idx, dtype=np.int32)
    emb_w = np.ascontiguousarray(np.asarray(inputs["emb_W"], dtype=f4))
    hidden = np.asarray(inputs["hidden"], dtype=f4).reshape(H)
    h_cols = np.ascontiguousarray(hidden.reshape(HC, 128).T)
    enc = np.asarray(inputs["encoder_outputs"], dtype=f4)
    enc0 = np.ascontiguousarray(enc[0:128])
    enc1 = np.ascontiguousarray(enc[128:L])
    attn_w = np.asarray(inputs["attn_W"], dtype=f4)  # [L, 2H]
    awt = np.ascontiguousarray(
        attn_w.T.reshape(CC, 128, L).transpose(1, 0, 2).reshape(128, CC * L)
    )
    attn_b = np.asarray(inputs["attn_b"], dtype=f4).reshape(1, L)
    comb_w = np.asarray(inputs["comb_W"], dtype=f4)  # [H, 2H]
    comb_b = np.asarray(inputs["comb_b"], dtype=f4)
    w_ih = np.asarray(inputs["w_ih"], dtype=f4)  # [3H, H]
    w_hh = np.asarray(inputs["w_hh"], dtype=f4)
    b_ih = np.asarray(inputs["b_ih"], dtype=f4)
    b_hh = np.asarray(inputs["b_hh"], dtype=f4)
    out_w = np.asarray(inputs["out_W"], dtype=f4)  # [V, H]
    out_b = np.asarray(inputs["out_b"], dtype=f4)

    bi_cols = np.ascontiguousarray(b_ih.reshape(GC, 128).T)
    bh_cols = np.ascontiguousarray(b_hh.reshape(GC, 128).T)

    if OUT_DT == "float32":
        w_np_dt = np.float32
    else:
        import ml_dtypes

        w_np_dt = getattr(ml_dtypes, OUT_DT)

    in_maps = []
    for k in range(NCORES):
        sl = slice(128 * k, 128 * (k + 1))
        h0s = np.ascontiguousarray(hidden[sl].reshape(128, 1))
        # comb_W rows-slice, chunk-packed for lhsT use
        cw_sl = comb_w[sl, :]  # [128(p), 2H(m)]
        cwt = np.ascontiguousarray(
            cw_sl.reshape(128, CC, 128).transpose(2, 1, 0).reshape(128, CC * 128)
        )
        cb = np.ascontiguousarray(comb_b[sl].reshape(128, 1))
        iwt = np.ascontiguousarray(w_ih[:, sl].T)  # [128(j), 3H(m)]
        hwt = np.ascontiguousarray(w_hh[:, sl].T)
        a, b = VS * k, min(VS * (k + 1), V)
        wpad = np.zeros((VS, H), dtype=f4)
        wpad[: b - a] = out_w[a:b]
        owt = np.ascontiguousarray(wpad.T).astype(w_np_dt)
        obp = np.full((1, VS), NEG_BIG, dtype=f4)
        obp[0, : b - a] = out_b[a:b]
        in_maps.append(
            {
                "tok2": tok2,
                "emb_w": emb_w,
                "h_cols": h_cols,
                "h0_slice": h0s,
                "enc0": enc0,
                "enc1": enc1,
                "attn_wt": awt,
                "attn_b": attn_b,
                "comb_wt": cwt,
                "comb_b": cb,
                "wih_t": iwt,
                "whh_t": hwt,
                "bi_cols": bi_cols,
                "bh_cols": bh_cols,
                "out_wt": owt,
                "out_b": obp,
            }
        )
    return in_maps


def _assemble(results):
    logp = np.concatenate([results[k]["logp_s"] for k in range(NCORES)])[:V]
    logp = logp.reshape(1, V).astype(np.float32)
    hcols = results[0]["hnew_cols"]  # [128, HC]
    h_new = np.ascontiguousarray(hcols.T).reshape(1, 1, H).astype(np.float32)
    aw = results[0]["aw_out"].reshape(1, L).astype(np.float32)
    return logp, h_new, aw


def _run(inputs, trace=False):
    from concourse.bass_utils import run_bass_kernel_spmd

    nc = _get_nc(OUT_DT, NBUFS)
    in_maps = _make_in_maps(inputs)
    res = run_bass_kernel_spmd(
        nc,
        in_maps,
        core_ids=list(range(NCORES)),
        trace=trace,
        trace_cores=list(range(NCORES)) if trace else None,
    )
    return _assemble(res.results), res


def kernel(**inputs):
    (logp, h_new, aw), _ = _run(inputs, trace=False)
    return logp, h_new, aw


def _make_timed_callable(nc, in_maps):
    """Mirror run_bass_via_pjrt's multi-core path, but keep inputs
    device-resident and return a re-invokable (fn, fresh_zeros) pair so
    repeated dispatches can be wall-clock timed."""
    import jax
    from jax.experimental.shard_map import shard_map
    from jax.sharding import Mesh, NamedSharding, PartitionSpec

    from concourse import bass2jax, mybir

    bass2jax.install_neuronx_cc_hook()
    n_cores = len(in_maps)
    partition_name = (
        nc.partition_id_tensor.name if nc.partition_id_tensor else None
    )
    in_names, out_names, out_avals, zero_outs = [], [], [], []
    for alloc in nc.m.functions[0].allocations:
        if not isinstance(alloc, mybir.MemoryLocationSet):
            continue
        name = alloc.memorylocations[0].name
        if alloc.kind == "ExternalInput":
            if name != partition_name:
                in_names.append(name)
        elif alloc.kind == "ExternalOutput":
            shape = tuple(alloc.tensor_shape)
            dtype = mybir.dt.np(alloc.dtype)
            out_names.append(name)
            out_avals.append(jax.core.ShapedArray(shape, dtype))
            zero_outs.append(np.zeros(shape, dtype))
    n_params = len(in_names)
    n_outs = len(out_avals)
    all_in_names = list(in_names) + list(out_names)
    if partition_name is not None:
        all_in_names.append(partition_name)
    donate = tuple(range(n_params, n_params + n_outs))

    def _body(*args):
        operands = list(args)
        if partition_name is not None:
            operands.append(bass2jax.partition_id_tensor())
        outs = bass2jax._bass_exec_p.bind(
            *operands,
            out_avals=tuple(out_avals),
            in_names=tuple(all_in_names),
            out_names=tuple(out_names),
            lowering_input_output_aliases=(),
            sim_require_finite=True,
            sim_require_nnan=True,
            nc=nc,
        )
        return tuple(outs)

    devices = jax.devices()[:n_cores]
    mesh = Mesh(np.asarray(devices), ("core",))
    spec = NamedSharding(mesh, PartitionSpec("core"))
    fn = jax.jit(
        shard_map(
            _body,
            mesh=mesh,
            in_specs=(PartitionSpec("core"),) * (n_params + n_outs),
            out_specs=(PartitionSpec("core"),) * n_outs,
            check_rep=False,
        ),
        donate_argnums=donate,
        keep_unused=True,
    )
    concat_in = [
        np.concatenate([np.asarray(in_maps[c][nm]) for c in range(n_cores)], axis=0)
        for nm in in_names
    ]
    dev_in = [jax.device_put(x, spec) for x in concat_in]

    def fresh_zeros():
        return [
            jax.device_put(
                np.zeros((n_cores * z.shape[0], *z.shape[1:]), z.dtype), spec
            )
            for z in zero_outs
        ]

    def call(zeros):
        out = fn(*dev_in, *zeros)
        jax.block_until_ready(out)
        return out

    return call, fresh_zeros, out_names, out_avals


def _build_baseline():
    """Tiny 8-core NEFF with the same dispatch structure, used to subtract
    RPC/dispatch overhead from wall-clock timings."""
    import concourse.bacc as bacc
    import concourse.tile as tile
    from concourse import mybir

    f32 = mybir.dt.float32
    nc = bacc.Bacc(
        "TRN2",
        target_bir_lowering=False,
        debug=False,
        enable_asserts=True,
        num_devices=NCORES,
    )
    x_d = nc.dram_tensor("x", [1, 4], f32, kind="ExternalInput").ap()
    y_d = nc.dram_tensor("y", [1, 4], f32, kind="ExternalOutput").ap()
    with tile.TileContext(nc) as tc:
        with tc.tile_pool(name="p", bufs=1) as pool:
            t = pool.tile([1, 4], f32, name="t")
            nc.sync.dma_start(out=t[:], in_=x_d[:, :])
            nc.sync.dma_start(out=y_d[:, :], in_=t[0:1, :])
    nc.compile()
    return nc


def time_kernel(inputs, iters=12):
    """Returns (outputs, est_exec_ns, t_full_ns, t_base_ns)."""
    import time as _time

    nc = _get_nc(OUT_DT, NBUFS)
    in_maps = _make_in_maps(inputs)
    call, fresh_zeros, out_names, _ = _make_timed_callable(nc, in_maps)

    out = call(fresh_zeros())  # warm-up + correctness output
    arrs = [np.asarray(a) for a in out]
    # arrs[i] has shape (NCORES*s0, ...): split back into per-core dicts
    results = []
    for c in range(NCORES):
        d = {}
        for i, nm in enumerate(out_names):
            a = arrs[i]
            s0 = a.shape[0] // NCORES
            d[nm] = a[c * s0 : (c + 1) * s0]
        results.append(d)

    zsets = [fresh_zeros() for _ in range(iters)]
    times = []
    for z in zsets:
        t0 = _time.perf_counter()
        call(z)
        times.append(_time.perf_counter() - t0)
    t_full = min(times)

    bnc = _build_baseline()
    bmaps = [{"x": np.zeros((1, 4), np.float32)} for _ in range(NCORES)]
    bcall, bzeros, _, _ = _make_timed_callable(bnc, bmaps)
    bcall(bzeros())
    bz = [bzeros() for _ in range(iters)]
    btimes = []
    for z in bz:
        t0 = _time.perf_counter()
        bcall(z)
        btimes.append(_time.perf_counter() - t0)
    t_base = min(btimes)

    est = max(t_full - t_base, 0.0)
    return _assemble(results), int(est * 1e9), int(t_full * 1e9), int(t_base * 1e9)


def run_traced(**inputs):
    try:
        outs, res = _run(inputs, trace=True)
        if res.exec_time_ns is not None:
            return outs, res.exec_time_ns
    except ModuleNotFoundError:
        pass
    outs, est, t_full, t_base = time_kernel(inputs)
    print(f"  (wall dispatch: full={t_full / 1e3:.0f}us base={t_base / 1e3:.0f}us)")
    return outs, est
